# revision 7
# baseline (speedup 1.0000x reference)
"""Trainium2 Bass kernel for a dense transformer block (nn_Block_52037823758381).

Sharding: data-parallel over batch (2 groups of 4 cores) x tensor-parallel
over heads / FFN hidden within each group. All matmuls run in float32r
(FP22 multiply, FP32 accumulate) which is 4x the FP32 rate on the PE.
"""

import os
from contextlib import ExitStack

import numpy as np

import concourse.bass as bass
import concourse.mybir as mybir
import concourse.tile as tile
from concourse.bass_utils import run_bass_kernel_spmd

F32 = mybir.dt.float32
F32R = mybir.dt.float32r
AF = mybir.ActivationFunctionType
ALU = mybir.AluOpType

P = 128
D = 2048
T = 2048
NH = 4        # heads per core
HS = 128
FFL = 2048    # FFN hidden per core
EPS = 1e-5
N_CORES = 8
GROUPS = [[0, 1, 2, 3], [4, 5, 6, 7]]
ISQ = 1.0 / np.sqrt(HS)
SHARD = T // 4  # 512 rows per core after reduce-scatter


# ---------------------------------------------------------------------------
# walrus in this toolchain accepts one sync wait per instruction; split the
# rest into single-wait NoOps in front of the instruction.
def _split_multi_waits(nc):
    counter = 0
    blocks = []
    for f in nc.m.functions:
        blocks.extend(f.blocks)
    for q in nc.m.queues:
        blocks.extend(q.blocks)
    for bb in blocks:
        changed = False
        new = []
        for ins in bb.instructions:
            si = ins.sync_info
            if (
                si is not None
                and len(si.on_wait) > 1
                and ins.engine is not None
                and ins.engine != mybir.EngineType.Unassigned
            ):
                waits = list(si.on_wait)
                for w in waits[:-1]:
                    nop = mybir.InstNoOp(name=f"I-waitsplit-{counter}")
                    counter += 1
                    nop.engine = ins.engine
                    nop.sync_info = mybir.SyncInfo(on_wait=[w], on_update=[])
                    new.append(nop)
                ins.sync_info = mybir.SyncInfo(
                    on_wait=waits[-1:], on_update=list(si.on_update)
                )
                changed = True
            new.append(ins)
        if changed:
            bb.instructions = new
    return counter


def _ln_normalize(nc, pool, stat, x_t, out_t):
    """out_t = (x_t - mean) * rsqrt(var + EPS), rowwise (free-dim reduce)."""
    st6 = stat.tile([P, 24], F32, name="st6", bufs=2)
    for g in range(4):
        nc.vector.bn_stats(
            st6[:, g * 6 : (g + 1) * 6], x_t[:, g * 512 : (g + 1) * 512]
        )
    aggr = stat.tile([P, 2], F32, name="aggr", bufs=2)
    nc.vector.bn_aggr(aggr[:], st6[:].rearrange("p (g f) -> p g f", f=6))
    epst = stat.tile([P, 1], F32, name="epst", bufs=2)
    nc.vector.memset(epst[:], EPS)
    std = stat.tile([P, 1], F32, name="std", bufs=2)
    nc.scalar.activation(std[:], aggr[:, 1:2], AF.Sqrt, bias=epst[:])
    rsq = stat.tile([P, 1], F32, name="rsq", bufs=2)
    nc.vector.reciprocal(rsq[:], std[:])
    nmr = stat.tile([P, 1], F32, name="nmr", bufs=2)
    nc.vector.tensor_scalar(
        nmr[:], aggr[:, 0:1], rsq[:], -1.0, ALU.mult, ALU.mult
    )
    nc.scalar.activation(out_t[:], x_t[:], AF.Identity, bias=nmr[:], scale=rsq[:])


def _build_program():
    nc = bass.Bass(trn_type="TRN2", num_devices=N_CORES)

    xp = nc.declare_dram_parameter("xp", [T, D], F32, isOutput=False)
    wq = nc.declare_dram_parameter("wq", [D, 512], F32, isOutput=False)
    wk = nc.declare_dram_parameter("wk", [D, 512], F32, isOutput=False)
    wv = nc.declare_dram_parameter("wv", [D, 512], F32, isOutput=False)
    bqkv = nc.declare_dram_parameter("bqkv", [3, 512], F32, isOutput=False)
    wp = nc.declare_dram_parameter("wp", [512, D], F32, isOutput=False)
    w1 = nc.declare_dram_parameter("w1", [D, FFL], F32, isOutput=False)
    b1 = nc.declare_dram_parameter("b1", [FFL], F32, isOutput=False)
    w2 = nc.declare_dram_parameter("w2", [FFL, D], F32, isOutput=False)
    masks = nc.declare_dram_parameter("masks", [4, P, 512], F32, isOutput=False)
    identity = nc.declare_dram_parameter("identity", [P, P], F32, isOutput=False)
    ones = nc.declare_dram_parameter("ones", [512, 1], F32, isOutput=False)
    out = nc.declare_dram_parameter("out", [SHARD, D], F32, isOutput=True)

    with tile.TileContext(nc) as tc, ExitStack() as es:
        cst = es.enter_context(tc.tile_pool(name="consts", bufs=1))
        stat = es.enter_context(tc.tile_pool(name="stats", bufs=1))
        dram = es.enter_context(tc.tile_pool(name="dram", bufs=1, space="DRAM"))

        ident = cst.tile([P, P], F32R, name="ident")
        nc.sync.dma_start(ident[:], identity[:].bitcast(F32R))
        ones_col = cst.tile([P, 1], F32R, name="ones_col")
        nc.sync.dma_start(ones_col[:], ones[:P].bitcast(F32R))
        ones_row = cst.tile([1, 512], F32R, name="ones_row")
        nc.sync.dma_start(ones_row[:], ones[:].rearrange("a b -> b a").bitcast(F32R))
        bq_sb = cst.tile([1, 512], F32R, name="bq")
        bk_sb = cst.tile([1, 512], F32R, name="bk")
        bv_sb = cst.tile([1, 512], F32R, name="bv")
        nc.sync.dma_start(bq_sb[:], bqkv[0:1].bitcast(F32R))
        nc.sync.dma_start(bk_sb[:], bqkv[1:2].bitcast(F32R))
        nc.sync.dma_start(bv_sb[:], bqkv[2:3].bitcast(F32R))
        b1_sb = cst.tile([P, FFL // P], F32, name="b1t")
        nc.sync.dma_start(b1_sb[:], b1.rearrange("(c p) -> p c", p=P))

        qd = dram.tile([NH * P, T], F32, name="qd")
        vd = dram.tile([T, 512], F32, name="vd")
        ar_in = dram.tile([T, D], F32, name="ar_in")
        ar_out = dram.tile([T, D], F32, name="ar_out")
        rs_in = dram.tile([T, D], F32, name="rs_in")
        rs_out = dram.tile([SHARD, D], F32, name="rs_out")

        es_attn = ExitStack()
        pat = es_attn.enter_context(tc.tile_pool(name="attnstore", bufs=1))
        attn_sb = [pat.tile([P, T], F32R, name=f"attn{h}") for h in range(NH)]

        es_qkv = ExitStack()
        pq = es_qkv.enter_context(tc.tile_pool(name="kstore", bufs=1))
        kT = [pq.tile([P, T], F32R, name=f"kT{h}") for h in range(NH)]

        # ---------------- Phase A: LN1 + transpose + QKV ----------------
        with (
            tc.tile_pool(name="phA", bufs=1) as pA,
            tc.tile_pool(name="phA_ps", bufs=1, space="PSUM") as psA,
        ):
            for rb in range(4):  # 512-row blocks
                r0 = rb * 512
                hT = [
                    pA.tile([P, 512], F32R, name=f"hT{d}", bufs=1) for d in range(16)
                ]
                for rt in range(4):
                    x_t = pA.tile([P, D], F32, name="x_t", bufs=2)
                    nc.sync.dma_start(x_t[:], xp[r0 + rt * P : r0 + (rt + 1) * P])
                    h_t = pA.tile([P, D], F32R, name="h_t", bufs=2)
                    _ln_normalize(nc, pA, stat, x_t, h_t)
                    for d in range(16):
                        tp = psA.tile([P, P], F32R, name="tp", bufs=2)
                        nc.tensor.transpose(
                            tp[:], h_t[:, d * P : (d + 1) * P], ident[:]
                        )
                        nc.scalar.copy(hT[d][:, rt * P : (rt + 1) * P], tp[:])

                # q and k passes (transposed outputs)
                for which, wsrc, bias_sb in (
                    ("q", wq, bq_sb),
                    ("k", wk, bk_sb),
                ):
                    ps = [
                        psA.tile([P, 512], F32, name=f"mm{cc}", bufs=1)
                        for cc in range(4)
                    ]
                    for d in range(16):
                        ws = pA.tile([P, 512], F32R, name="wstrip", bufs=3)
                        nc.sync.dma_start(
                            ws[:], wsrc[d * P : (d + 1) * P].bitcast(F32R)
                        )
                        for cc in range(4):
                            nc.tensor.matmul(
                                ps[cc][:],
                                ws[:, cc * P : (cc + 1) * P],
                                hT[d][:],
                                start=(d == 0),
                                stop=False,
                            )
                    for cc in range(4):
                        nc.tensor.matmul(
                            ps[cc][:],
                            bias_sb[0:1, cc * P : (cc + 1) * P],
                            ones_row[0:1, :],
                            start=False,
                            stop=True,
                        )
                        if which == "k":
                            nc.scalar.copy(kT[cc][:, r0 : r0 + 512], ps[cc][:])
                        else:
                            qstg = pA.tile([P, 512], F32, name="qstg", bufs=3)
                            nc.scalar.copy(qstg[:], ps[cc][:])
                            nc.sync.dma_start(
                                qd[cc * P : (cc + 1) * P, r0 : r0 + 512], qstg[:]
                            )

                # v pass (natural layout)
                psv = [
                    psA.tile([P, 512], F32, name=f"mm{rt}", bufs=1)
                    for rt in range(4)
                ]
                for d in range(16):
                    ws = pA.tile([P, 512], F32R, name="wstrip", bufs=3)
                    nc.sync.dma_start(ws[:], wv[d * P : (d + 1) * P].bitcast(F32R))
                    for rt in range(4):
                        nc.tensor.matmul(
                            psv[rt][:],
                            hT[d][:, rt * P : (rt + 1) * P],
                            ws[:],
                            start=(d == 0),
                            stop=False,
                        )
                for rt in range(4):
                    nc.tensor.matmul(
                        psv[rt][:],
                        ones_row[0:1, :P],
                        bv_sb[0:1, :],
                        start=False,
                        stop=True,
                    )
                    vstg = pA.tile([P, 512], F32, name="vstg", bufs=3)
                    nc.scalar.copy(vstg[:], psv[rt][:])
                    nc.sync.dma_start(
                        vd[r0 + rt * P : r0 + (rt + 1) * P, :], vstg[:]
                    )

        # ---------------- Phase B: attention ----------------
        with (
            tc.tile_pool(name="phB", bufs=1) as pB,
            tc.tile_pool(name="phB_ps", bufs=1, space="PSUM") as psB,
        ):
            mask_sb = []
            for i in range(4):
                m = pB.tile([P, 512], F32, name=f"mask{i}")
                nc.sync.dma_start(m[:], masks[i])
                mask_sb.append(m)
            for lh in range(NH):
                for qg in range(4):
                    q0 = qg * 512
                    nkb = 4 * qg + 4
                    q_t = pB.tile([P, 512], F32R, name="q_t", bufs=2)
                    nc.sync.dma_start(
                        q_t[:],
                        qd[lh * P : (lh + 1) * P, q0 : q0 + 512].bitcast(F32R),
                    )
                    att_ps = psB.tile([P, 512], F32, name="att_ps", bufs=1)
                    den_ps = psB.tile([1, 512], F32, name="den_ps", bufs=1)
                    for kb in range(nkb):
                        sc = psB.tile([P, 512], F32, name="sc", bufs=2)
                        nc.tensor.matmul(
                            sc[:],
                            kT[lh][:, kb * P : (kb + 1) * P],
                            q_t[:],
                            start=True,
                            stop=True,
                        )
                        ex = pB.tile([P, 512], F32R, name="ex", bufs=3)
                        nc.scalar.activation(ex[:], sc[:], AF.Exp, scale=float(ISQ))
                        if kb >= 4 * qg:
                            nc.vector.tensor_mul(
                                ex[:], ex[:], mask_sb[kb - 4 * qg][:]
                            )
                        v_t = pB.tile([P, P], F32R, name="v_t", bufs=3)
                        nc.sync.dma_start(
                            v_t[:],
                            vd[
                                kb * P : (kb + 1) * P, lh * P : (lh + 1) * P
                            ].bitcast(F32R),
                        )
                        nc.tensor.matmul(
                            att_ps[:],
                            v_t[:],
                            ex[:],
                            start=(kb == 0),
                            stop=(kb == nkb - 1),
                        )
                        nc.tensor.matmul(
                            den_ps[:],
                            ones_col[:],
                            ex[:],
                            start=(kb == 0),
                            stop=(kb == nkb - 1),
                        )
                    rec = pB.tile([1, 512], F32R, name="rec", bufs=2)
                    with nc.allow_low_precision(reason="softmax reciprocal f32r"):
                        nc.vector.reciprocal(rec[:], den_ps[:])
                    bc_ps = psB.tile([P, 512], F32, name="bc_ps", bufs=1)
                    nc.tensor.matmul(
                        bc_ps[:], ones_row[0:1, :P], rec[:], start=True, stop=True
                    )
                    bc = pB.tile([P, 512], F32, name="bc", bufs=2)
                    nc.scalar.copy(bc[:], bc_ps[:])
                    nc.vector.tensor_mul(
                        attn_sb[lh][:, q0 : q0 + 512], att_ps[:], bc[:]
                    )

        es_qkv.close()

        # ---------------- Phase C: proj + AllReduce ----------------
        with (
            tc.tile_pool(name="phC", bufs=1) as pC,
            tc.tile_pool(name="phC_ps", bufs=1, space="PSUM") as psC,
        ):
            wp_sb = []
            for lh in range(NH):
                w = pC.tile([P, D], F32R, name=f"wp{lh}")
                nc.sync.dma_start(w[:], wp[lh * P : (lh + 1) * P].bitcast(F32R))
                wp_sb.append(w)
            for rc in range(16):
                for dc in range(4):
                    pp = psC.tile([P, 512], F32, name="pp", bufs=2)
                    for lh in range(NH):
                        nc.tensor.matmul(
                            pp[:],
                            attn_sb[lh][:, rc * P : (rc + 1) * P],
                            wp_sb[lh][:, dc * 512 : (dc + 1) * 512],
                            start=(lh == 0),
                            stop=(lh == NH - 1),
                        )
                    ev = pC.tile([P, 512], F32, name="ev", bufs=3)
                    nc.scalar.copy(ev[:], pp[:])
                    nc.sync.dma_start(
                        ar_in[rc * P : (rc + 1) * P, dc * 512 : (dc + 1) * 512],
                        ev[:],
                    )
            nc.gpsimd.collective_compute(
                "AllReduce",
                ALU.add,
                replica_groups=GROUPS,
                ins=[ar_in.opt()],
                outs=[ar_out.opt()],
            )

        es_attn.close()

        # ---------------- Phase D: FFN + ReduceScatter ----------------
        with (
            tc.tile_pool(name="phD", bufs=1) as pD,
            tc.tile_pool(name="phD_ps", bufs=1, space="PSUM") as psD,
        ):
            for sbi in range(2):  # 1024-row super-blocks
                s0 = sbi * 1024
                ln2T = [
                    pD.tile([P, 1024], F32R, name=f"l2T{d}", bufs=1)
                    for d in range(16)
                ]
                g1T = [
                    pD.tile([P, 1024], F32R, name=f"g1T{f}", bufs=1)
                    for f in range(16)
                ]
                for rt in range(8):
                    r0 = s0 + rt * P
                    x2_t = pD.tile([P, D], F32, name="x2t", bufs=1)
                    nc.sync.dma_start(x2_t[:], xp[r0 : r0 + P])
                    nc.gpsimd.dma_start(
                        x2_t[:], ar_out[r0 : r0 + P], accum_op=ALU.add
                    )
                    q25 = pD.tile([P, D], F32, name="q25", bufs=1)
                    nc.scalar.activation(q25[:], x2_t[:], AF.Copy, scale=0.25)
                    nc.sync.dma_start(rs_in[r0 : r0 + P], q25[:])
                    l2h = pD.tile([P, D], F32R, name="l2h", bufs=1)
                    _ln_normalize(nc, pD, stat, x2_t, l2h)
                    for d in range(16):
                        tp = psD.tile(
                            [P, P], F32R, name="tp2", tag=f"ffps{d % 2}", bufs=1
                        )
                        nc.tensor.transpose(
                            tp[:], l2h[:, d * P : (d + 1) * P], ident[:]
                        )
                        nc.scalar.copy(ln2T[d][:, rt * P : (rt + 1) * P], tp[:])

                # h1 = gelu(ln2T.T @ w1 + b1), produced transposed
                for ffg in range(4):
                    h1ps = [
                        psD.tile([P, 512], F32, name=f"h1ps{j}", tag=f"ffps{j}", bufs=1)
                        for j in range(8)
                    ]
                    for d in range(16):
                        w1s = pD.tile([P, 512], F32R, name="w1s", bufs=3)
                        nc.sync.dma_start(
                            w1s[:],
                            w1[
                                d * P : (d + 1) * P, ffg * 512 : (ffg + 1) * 512
                            ].bitcast(F32R),
                        )
                        for f4 in range(4):
                            for nb in range(2):
                                nc.tensor.matmul(
                                    h1ps[f4 * 2 + nb][:],
                                    w1s[:, f4 * P : (f4 + 1) * P],
                                    ln2T[d][:, nb * 512 : (nb + 1) * 512],
                                    start=(d == 0),
                                    stop=(d == 15),
                                )
                    for f4 in range(4):
                        ff = ffg * 4 + f4
                        for nb in range(2):
                            nc.scalar.activation(
                                g1T[ff][:, nb * 512 : (nb + 1) * 512],
                                h1ps[f4 * 2 + nb][:],
                                AF.Gelu,
                                bias=b1_sb[:, ff : ff + 1],
                            )

                # h2 = g1T.T @ w2, evicted with += into rs_in (x2/4 pre-written)
                for dc in range(4):
                    h2ps = [
                        psD.tile([P, 512], F32, name=f"h2ps{rc}", tag=f"ffps{rc}", bufs=1)
                        for rc in range(8)
                    ]
                    for half in range(2):
                        w2s = []
                        for j in range(8):
                            w = pD.tile([P, 512], F32R, name=f"w2s{j}", bufs=1)
                            nc.sync.dma_start(
                                w[:],
                                w2[
                                    (half * 8 + j) * P : (half * 8 + j + 1) * P,
                                    dc * 512 : (dc + 1) * 512,
                                ].bitcast(F32R),
                            )
                            w2s.append(w)
                        for rc in range(8):
                            for j in range(8):
                                nc.tensor.matmul(
                                    h2ps[rc][:],
                                    g1T[half * 8 + j][:, rc * P : (rc + 1) * P],
                                    w2s[j][:],
                                    start=(half == 0 and j == 0),
                                    stop=(half == 1 and j == 7),
                                )
                    for rc in range(8):
                        ev2 = pD.tile([P, 512], F32, name="ev2", bufs=3)
                        nc.scalar.copy(ev2[:], h2ps[rc][:])
                        nc.gpsimd.dma_start(
                            rs_in[
                                s0 + rc * P : s0 + (rc + 1) * P,
                                dc * 512 : (dc + 1) * 512,
                            ],
                            ev2[:],
                            accum_op=ALU.add,
                        )

            nc.gpsimd.collective_compute(
                "ReduceScatter",
                ALU.add,
                replica_groups=GROUPS,
                ins=[rs_in.opt()],
                outs=[rs_out.opt()],
            )
            nc.sync.dma_start(out[:], rs_out[:])

    _split_multi_waits(nc)
    return nc


_program = None


def _get_program():
    global _program
    if _program is None:
        _program = _build_program()
    return _program


def kernel(
    x,
    ln1_g,
    ln1_b,
    W_attn,
    b_attn,
    W_proj,
    b_proj,
    ln2_g,
    ln2_b,
    W1,
    b1,
    W2,
    b2,
):
    x = np.asarray(x, np.float32)
    W_attn_eff = np.asarray(ln1_g, np.float32)[:, None] * np.asarray(W_attn, np.float32)
    b_attn_eff = np.asarray(b_attn, np.float32) + np.asarray(
        ln1_b, np.float32
    ) @ np.asarray(W_attn, np.float32)
    W1_eff = np.asarray(ln2_g, np.float32)[:, None] * np.asarray(W1, np.float32)
    b1_eff = np.asarray(b1, np.float32) + np.asarray(ln2_b, np.float32) @ np.asarray(
        W1, np.float32
    )
    W_proj = np.asarray(W_proj, np.float32)
    W2 = np.asarray(W2, np.float32)
    b_proj = np.asarray(b_proj, np.float32)

    # causal masks for the 4 diagonal-block alignments
    mk = np.zeros((4, P, 512), np.float32)
    jj = np.arange(512)[None, :]
    pp = np.arange(P)[:, None]
    for i in range(4):
        mk[i] = (i * P + pp <= jj).astype(np.float32)
    ident = np.eye(P, dtype=np.float32)
    ones = np.ones((512, 1), np.float32)

    in_maps = []
    for core in range(N_CORES):
        b = core // 4
        r = core % 4
        cs = slice(512 * r, 512 * (r + 1))
        fs = slice(FFL * r, FFL * (r + 1))
        in_maps.append(
            {
                "xp": np.ascontiguousarray(x[b]) + b_proj,
                "wq": np.ascontiguousarray(W_attn_eff[:, cs]),
                "wk": np.ascontiguousarray(W_attn_eff[:, D + 512 * r : D + 512 * (r + 1)]),
                "wv": np.ascontiguousarray(
                    W_attn_eff[:, 2 * D + 512 * r : 2 * D + 512 * (r + 1)]
                ),
                "bqkv": np.stack(
                    [
                        b_attn_eff[cs],
                        b_attn_eff[D + 512 * r : D + 512 * (r + 1)],
                        b_attn_eff[2 * D + 512 * r : 2 * D + 512 * (r + 1)],
                    ]
                ).astype(np.float32),
                "wp": np.ascontiguousarray(W_proj[cs, :]),
                "w1": np.ascontiguousarray(W1_eff[:, fs]),
                "b1": np.ascontiguousarray(b1_eff[fs]),
                "w2": np.ascontiguousarray(W2[fs, :]),
                "masks": mk,
                "identity": ident,
                "ones": ones,
            }
        )

    nc = _get_program()
    res = run_bass_kernel_spmd(
        nc,
        in_maps,
        list(range(N_CORES)),
        trace=bool(os.environ.get("KERNEL_TRACE")),
    )
    kernel.last_results = res

    outb = []
    for b in range(2):
        shards = [res.results[4 * b + r]["out"] for r in range(4)]
        outb.append(np.concatenate(shards, axis=0))
    full = np.stack(outb).astype(np.float32)
    full = full + np.asarray(b2, np.float32)
    return full.reshape(2, T, D)
